# revision 44
# baseline (speedup 1.0000x reference)
"""Trainium2 Bass kernel for CustomBCELoss.

Reference semantics (per torch BCELoss with per-channel weighting):
    p, t flattened channel-first to (C=3, M=8388608)
    ones[c]   = count_nonzero(t[c])
    weight[c] = M / max(ones[c], 1)  if ones[c] > 0 else 1000.0
    bce[c]    = -mean(t*max(log p, -100) + (1-t)*max(log1p(-p), -100))
    out       = mean(weight * bce)

Since t ∈ {0,1}, the per-element term is log|p + t - 1|, and with
p ∈ [1e-4, 1-1e-4] (post-sigmoid probabilities) the -100 clamp never
fires: |p + t - 1| >= ~6e-5 so log >= ~-10.

8-way data-parallel over the flat element range. Per-core pipeline over
[128, f] tiles. The stream is DMA-bound: 16 HW DMA engines sustain
~425 GB/s/core aggregate, so the 25.2 MB p+t stream floors at ~59 us.
The schedule is built so nothing else ever gates that stream:
  * tiles ramp 512/512/1024 at the start (first compute starts ~1.2 us
    after the stream does, instead of waiting ~13 us for a 4 MB
    4096-col tile pair), cruise at 2048, taper 1024/512/512 at the end
    so the post-stream drain is one short STT+TT+Ln chain;
  * all tiles are <= 2048 cols -> 1 MB pool slots, bufs=8 per pool
    (16 MB SBUF) gives the in-order Sync issuer ~8 tiles of runway so
    slot recycling never stalls descriptor issue;
  * engine split per tile: Sync issues all DMA; PE does per-segment
    positive counts (t is exactly 0.0f/1.0f, so the strided bf16 view
    of its high 2 bytes is exactly 0.0/1.0 -- single-pass bf16 matmuls
    ones.T @ t into PSUM, exact); DVE computes d = (p - 1) + t (fused
    STT) and u = d*d for SQUARE_TILES (their Ln accumulates 2*log|d|,
    halved on the host); ACT does u = |d| for the remaining tiles and
    Ln(u) in place with fused per-partition accum_out. The square/abs
    split (~46% squared) balances DVE and ACT at ~0.7 duty each under
    the 425 GB/s stream. A dummy Ln in the preamble pins the
    natural_log table set (contains Abs+Ln): one load, preamble.
Tiles never cross an (n, c) half-block boundary, so per-tile/per-segment
partials map 1:1 to channels on the host, which applies the tiny
weight/mean epilogue in float64.
"""

import numpy as np

import concourse.bacc as bacc
import concourse.bass as bass
import concourse.tile as tile
from concourse import mybir
from concourse.bass_utils import run_bass_kernel_spmd

N_CORES = 8
C = 3
SPATIAL = 128 * 128 * 128            # elements per (n, c) block
N_BATCH = 4
FULL = N_BATCH * C * SPATIAL         # 25_165_824 total elements
PER_CORE = FULL // N_CORES           # 3_145_728
P = 128
# Per-partition column counts per tile; sum must equal PER_CORE / P = 24576.
# Mild ramp at the start (a string of small tiles would starve the 16 DMA
# engines: each Sync DMA_DIRECT2D issue costs ~0.6 us, so sub-1MB tiles
# can't keep 425 GB/s fed), cruise at 2048, taper at the end (short drain).
TILE_F = [1024, 2048, 2048, 2048, 1024,
          2048, 2048, 2048, 2048,
          2048, 2048, 1024, 1024, 1024, 512, 512]
NTILES = len(TILE_F)
TILE_ELEMS = [P * f for f in TILE_F]
assert sum(TILE_ELEMS) == PER_CORE
# |d| as d*d on DVE for these tiles (rest: Abs on ACT). Placement sets
# each engine's backlog at stream end: tiles 5-8 all abs lets DVE catch
# up mid-stream; tile 9 square lets ACT recover; the taper alternates
# square/abs so BOTH engines share the drain (an all-square taper
# saturates DVE with a serial STT->TT ladder, an all-abs taper does the
# same to ACT with Abs->Ln — each measured ~+4 us).
# CAUTION: strict square/abs alternation across ALL tiles (e.g. evens
# square) makes the NEFF run with every engine clock derated 1.2x
# (+5 us) — some static power model keys on sustained DVE+ACT
# co-activity. Alternation on a handful of tiles with clusters
# elsewhere measures at full clock.
SQUARE_TILES = {0, 2, 4, 9, 11, 13, 15}
HALF_BLOCK_COLS = (SPATIAL // 2) // P          # 8192 cols per half-block
N_SEG = (PER_CORE // P) // HALF_BLOCK_COLS     # 3 segments per core
MM_N = 512                                      # matmul moving free dim
M_PER_CH = FULL // C                 # 8_388_608
EMPTY_WEIGHT = 1000.0

_NC_CACHE = None


def _build_nc():
    nc = bacc.Bacc(
        "TRN2", target_bir_lowering=False, debug=False, num_devices=N_CORES
    )
    p_in = nc.declare_dram_parameter(
        "p_in", [PER_CORE], mybir.dt.float32, isOutput=False
    )
    t_in = nc.declare_dram_parameter(
        "t_in", [PER_CORE], mybir.dt.float32, isOutput=False
    )
    vsum_out = nc.declare_dram_parameter(
        "vsum", [P, NTILES], mybir.dt.float32, isOutput=True
    )
    tsum_out = nc.declare_dram_parameter(
        "tsum", [1, N_SEG * MM_N], mybir.dt.float32, isOutput=True
    )

    seg_of_tile = []
    off = 0
    for f in TILE_F:
        assert off // HALF_BLOCK_COLS == (off + f - 1) // HALF_BLOCK_COLS
        seg_of_tile.append(off // HALF_BLOCK_COLS)
        off += f
    mm_total = {s: 0 for s in range(N_SEG)}
    for i, f in enumerate(TILE_F):
        mm_total[seg_of_tile[i]] += max(1, f // MM_N)

    with tile.TileContext(nc) as tc:
        with (
            tc.tile_pool(name="pp", bufs=10) as p_pool,
            tc.tile_pool(name="tp", bufs=10) as t_pool,
            tc.tile_pool(name="res", bufs=1) as res_pool,
            tc.tile_pool(name="ps", bufs=1, space="PSUM") as ps_pool,
        ):
            ones_t = res_pool.tile([P, 1], mybir.dt.bfloat16)
            nc.vector.memset(ones_t, 1.0)
            vsum_t = res_pool.tile([P, NTILES], mybir.dt.float32)
            cnt_sb = res_pool.tile([1, N_SEG * MM_N], mybir.dt.float32)
            # Dummy Ln pins the natural_log table set (contains Abs too).
            warm_t = res_pool.tile([P, 1], mybir.dt.float32)
            nc.vector.memset(warm_t, 1.0)
            nc.scalar.activation(
                out=warm_t, in_=warm_t, func=mybir.ActivationFunctionType.Ln
            )
            psum_seg = [
                ps_pool.tile(
                    [1, MM_N], mybir.dt.float32, tag=f"seg{s}", name=f"psum_seg{s}"
                )
                for s in range(N_SEG)
            ]
            mm_done = {s: 0 for s in range(N_SEG)}
            off = 0
            for i, f in enumerate(TILE_F):
                n = P * f
                p_src = p_in[off : off + n].rearrange("(p f) -> p f", p=P)
                t_src = t_in[off : off + n].rearrange("(p f) -> p f", p=P)
                off += n
                s = seg_of_tile[i]
                p_t = p_pool.tile([P, f], mybir.dt.float32, tag="p")
                t_t = t_pool.tile([P, f], mybir.dt.float32, tag="t")
                nc.sync.dma_start(out=p_t, in_=p_src)
                nc.sync.dma_start(out=t_t, in_=t_src)
                t_hi = t_t[:].bitcast(mybir.dt.bfloat16).rearrange(
                    "p (f two) -> p f two", two=2
                )[:, :, 1]
                # Sub-512 tiles contribute one narrow chunk accumulated
                # into the low lanes of the segment bank — the host sums
                # all 512 lanes, so lane balance doesn't matter.
                w = min(MM_N, f)
                for j in range(max(1, f // MM_N)):
                    nc.tensor.matmul(
                        out=psum_seg[s][:, :w],
                        lhsT=ones_t[:, :],
                        rhs=t_hi[:, j * w : (j + 1) * w],
                        start=(mm_done[s] == 0),
                        stop=(mm_done[s] == mm_total[s] - 1),
                    )
                    mm_done[s] += 1
                # d = (p - 1) + t, in place into p_t
                nc.vector.scalar_tensor_tensor(
                    out=p_t,
                    in0=p_t,
                    scalar=1.0,
                    in1=t_t,
                    op0=mybir.AluOpType.subtract,
                    op1=mybir.AluOpType.add,
                )
                if i in SQUARE_TILES:
                    nc.vector.tensor_tensor(
                        out=p_t, in0=p_t, in1=p_t, op=mybir.AluOpType.mult
                    )
                else:
                    nc.scalar.activation(
                        out=p_t, in_=p_t, func=mybir.ActivationFunctionType.Abs
                    )
                nc.scalar.activation(
                    out=p_t,
                    in_=p_t,
                    func=mybir.ActivationFunctionType.Ln,
                    accum_out=vsum_t[:, i : i + 1],
                )
            for s in range(N_SEG):
                if s == N_SEG - 1:
                    # seg2's counts become ready right as DVE runs the
                    # last tiles' STT/TT ladder — a DVE copy wedges into
                    # that in-order chain (+0.7 us measured). ACT has an
                    # idle window there; Copy is in every table set.
                    nc.scalar.activation(
                        out=cnt_sb[:, s * MM_N : (s + 1) * MM_N],
                        in_=psum_seg[s],
                        func=mybir.ActivationFunctionType.Copy,
                    )
                else:
                    nc.vector.tensor_copy(
                        out=cnt_sb[:, s * MM_N : (s + 1) * MM_N],
                        in_=psum_seg[s],
                    )
            # NOTE: DMA cannot read PSUM (dma_start asserts SBUF/DRAM
            # source), so counts bounce through SBUF on DVE. Collapsing
            # vsum's partition dim with an fp32 PE matmul looks
            # attractive (one 60B descriptor instead of 128 tiny ones)
            # but any fp32r matmul in the NEFF statically derates ALL
            # engine clocks by 1.2x (+5 us). Don't.
            # Ship results in readiness order so only a tiny vsum chunk
            # trails the last Ln: bulk vsum columns (ready mid-taper),
            # then counts, then the last taper columns.
            vs_split = NTILES - 3
            nc.sync.dma_start(
                out=vsum_out[:, :vs_split], in_=vsum_t[:, :vs_split]
            )
            nc.sync.dma_start(out=tsum_out[:], in_=cnt_sb)
            nc.sync.dma_start(
                out=vsum_out[:, vs_split:], in_=vsum_t[:, vs_split:]
            )
    nc.compile()
    return nc


def _get_nc():
    global _NC_CACHE
    if _NC_CACHE is None:
        _NC_CACHE = _build_nc()
    return _NC_CACHE


def _run_device(input, target, **spmd_kwargs):
    p_flat = np.ascontiguousarray(input, dtype=np.float32).reshape(-1)
    t_flat = np.ascontiguousarray(target, dtype=np.float32).reshape(-1)
    in_maps = []
    for k in range(N_CORES):
        sl = slice(k * PER_CORE, (k + 1) * PER_CORE)
        in_maps.append({"p_in": p_flat[sl], "t_in": t_flat[sl]})
    return run_bass_kernel_spmd(nc=_get_nc(), in_maps=in_maps,
                                core_ids=list(range(N_CORES)), **spmd_kwargs)


def _epilogue(results):
    sum_v = np.zeros(C, dtype=np.float64)
    sum_t = np.zeros(C, dtype=np.float64)
    for k in range(N_CORES):
        vs = results[k]["vsum"].astype(np.float64)   # [P, NTILES]
        ts = results[k]["tsum"].astype(np.float64)   # [1, N_SEG*MM_N]
        off = 0
        for i, n in enumerate(TILE_ELEMS):
            g = k * PER_CORE + off
            off += n
            ch = (g // SPATIAL) % C
            scale = 0.5 if i in SQUARE_TILES else 1.0
            sum_v[ch] += scale * vs[:, i].sum()
        for s in range(N_SEG):
            ch = ((k * N_SEG + s) // 2) % C
            sum_t[ch] += ts[0, s * MM_N : (s + 1) * MM_N].sum()
    total = float(M_PER_CH)
    ones = sum_t
    weight = np.where(ones > 0, total / np.maximum(ones, 1.0), EMPTY_WEIGHT)
    bce = -sum_v / total
    return np.asarray((weight * bce).mean(), dtype=np.float32)


def kernel(input, target):
    res = _run_device(input, target)
    return _epilogue(res.results)


# revision 46
# speedup vs baseline: 1.0954x; 1.0954x over previous
"""Trainium2 Bass kernel for CustomBCELoss.

Reference semantics (per torch BCELoss with per-channel weighting):
    p, t flattened channel-first to (C=3, M=8388608)
    ones[c]   = count_nonzero(t[c])
    weight[c] = M / max(ones[c], 1)  if ones[c] > 0 else 1000.0
    bce[c]    = -mean(t*max(log p, -100) + (1-t)*max(log1p(-p), -100))
    out       = mean(weight * bce)

Since t ∈ {0,1}, the per-element term is log|p + t - 1|, and with
p ∈ [1e-4, 1-1e-4] (post-sigmoid probabilities) the -100 clamp never
fires: |p + t - 1| >= ~6e-5 so log >= ~-10.

8-way data-parallel over the flat element range. Per-core pipeline over
[128, f] tiles. The stream is DMA-bound: 16 HW DMA engines sustain
~425 GB/s/core when the 8 cores' streams de-overlap (and chip-HBM
fair-share ~360 GB/s when they fully overlap — the run-to-run spread),
so the 25.2 MB p+t stream floors at ~59-70 us. The schedule is built so
nothing else ever gates that stream:
  * tiles open with 1024 (first compute ~2.5 us after the stream
    starts; a string of sub-1MB tiles would starve the engines — each
    Sync DMA_DIRECT2D issue costs ~0.6 us), cruise at 2048, taper
    1024/1024/1024/512/512 so the post-stream drain is short chains on
    small tiles. 256-col taper tiles were tried and are NET-WORSE (the
    two extra issue/semaphore sets cost more than the shorter chain).
  * all tiles are <= 2048 cols -> 1 MB pool slots, bufs=8 per pool
    (16 MB SBUF) gives the in-order Sync issuer ~8 tiles of runway so
    slot recycling never stalls descriptor issue;
  * engine split per tile: Sync issues all DMA; PE does per-segment
    positive counts (t is exactly 0.0f/1.0f, so the strided bf16 view
    of its high 2 bytes is exactly 0.0/1.0 -- single-pass bf16 matmuls
    ones.T @ t into PSUM, exact); DVE computes d = (p - 1) + t (fused
    STT) and u = d*d for SQUARE_TILES (their Ln accumulates 2*log|d|,
    halved on the host); ACT does u = |d| for the remaining tiles and
    Ln(u) in place with fused per-partition accum_out. See the
    SQUARE_TILES comment for the placement rules (engine backlog at
    stream end + the 1.2x clock-derate trap). A dummy Ln in the
    preamble pins the natural_log table set (contains Abs+Square+Ln):
    one load, preamble.
Tiles never cross an (n, c) half-block boundary, so per-tile/per-segment
partials map 1:1 to channels on the host, which applies the tiny
weight/mean epilogue in float64.
"""

import numpy as np

import concourse.bacc as bacc
import concourse.bass as bass
import concourse.tile as tile
from concourse import mybir
from concourse.bass_utils import run_bass_kernel_spmd

N_CORES = 8
C = 3
SPATIAL = 128 * 128 * 128            # elements per (n, c) block
N_BATCH = 4
FULL = N_BATCH * C * SPATIAL         # 25_165_824 total elements
PER_CORE = FULL // N_CORES           # 3_145_728
P = 128
# Per-partition column counts per tile; sum must equal PER_CORE / P = 24576.
# Mild ramp at the start (a string of small tiles would starve the 16 DMA
# engines: each Sync DMA_DIRECT2D issue costs ~0.6 us, so sub-1MB tiles
# can't keep 425 GB/s fed), cruise at 2048, taper at the end (short drain).
TILE_F = [1024, 2048, 2048, 2048, 1024,
          2048, 2048, 2048, 2048,
          2048, 2048, 1024, 1024, 1024, 512, 512]
NTILES = len(TILE_F)
TILE_ELEMS = [P * f for f in TILE_F]
assert sum(TILE_ELEMS) == PER_CORE
# |d| as d*d on DVE for these tiles (rest: Abs on ACT). Placement sets
# each engine's backlog at stream end: tiles 5-8 all abs lets DVE catch
# up mid-stream; tile 9 square lets ACT recover; the taper alternates
# square/abs so BOTH engines share the drain (an all-square taper
# saturates DVE with a serial STT->TT ladder, an all-abs taper does the
# same to ACT with Abs->Ln — each measured ~+4 us).
# CAUTION: strict square/abs alternation across ALL tiles (e.g. evens
# square) makes the NEFF run with every engine clock derated 1.2x
# (+5 us) — some static power model keys on sustained DVE+ACT
# co-activity. Alternation on a handful of tiles with clusters
# elsewhere measures at full clock.
SQUARE_TILES = {0, 2, 4, 9, 11, 13, 15}
HALF_BLOCK_COLS = (SPATIAL // 2) // P          # 8192 cols per half-block
N_SEG = (PER_CORE // P) // HALF_BLOCK_COLS     # 3 segments per core
MM_N = 512                                      # matmul moving free dim
M_PER_CH = FULL // C                 # 8_388_608
EMPTY_WEIGHT = 1000.0

_NC_CACHE = None


def _build_nc():
    nc = bacc.Bacc(
        "TRN2", target_bir_lowering=False, debug=False, num_devices=N_CORES
    )
    p_in = nc.declare_dram_parameter(
        "p_in", [PER_CORE], mybir.dt.float32, isOutput=False
    )
    t_in = nc.declare_dram_parameter(
        "t_in", [PER_CORE], mybir.dt.float32, isOutput=False
    )
    vsum_out = nc.declare_dram_parameter(
        "vsum", [P, NTILES], mybir.dt.float32, isOutput=True
    )
    tsum_out = nc.declare_dram_parameter(
        "tsum", [1, N_SEG * MM_N], mybir.dt.float32, isOutput=True
    )

    seg_of_tile = []
    off = 0
    for f in TILE_F:
        assert off // HALF_BLOCK_COLS == (off + f - 1) // HALF_BLOCK_COLS
        seg_of_tile.append(off // HALF_BLOCK_COLS)
        off += f
    mm_total = {s: 0 for s in range(N_SEG)}
    for i, f in enumerate(TILE_F):
        mm_total[seg_of_tile[i]] += max(1, f // MM_N)

    with tile.TileContext(nc) as tc:
        with (
            tc.tile_pool(name="pp", bufs=8) as p_pool,
            tc.tile_pool(name="tp", bufs=8) as t_pool,
            tc.tile_pool(name="res", bufs=1) as res_pool,
            tc.tile_pool(name="ps", bufs=1, space="PSUM") as ps_pool,
        ):
            ones_t = res_pool.tile([P, 1], mybir.dt.bfloat16)
            nc.vector.memset(ones_t, 1.0)
            vsum_t = res_pool.tile([P, NTILES], mybir.dt.float32)
            cnt_sb = res_pool.tile([1, N_SEG * MM_N], mybir.dt.float32)
            # Dummy Ln pins the natural_log table set (contains Abs too).
            warm_t = res_pool.tile([P, 1], mybir.dt.float32)
            nc.vector.memset(warm_t, 1.0)
            nc.scalar.activation(
                out=warm_t, in_=warm_t, func=mybir.ActivationFunctionType.Ln
            )
            psum_seg = [
                ps_pool.tile(
                    [1, MM_N], mybir.dt.float32, tag=f"seg{s}", name=f"psum_seg{s}"
                )
                for s in range(N_SEG)
            ]
            mm_done = {s: 0 for s in range(N_SEG)}
            off = 0
            for i, f in enumerate(TILE_F):
                n = P * f
                p_src = p_in[off : off + n].rearrange("(p f) -> p f", p=P)
                t_src = t_in[off : off + n].rearrange("(p f) -> p f", p=P)
                off += n
                s = seg_of_tile[i]
                p_t = p_pool.tile([P, f], mybir.dt.float32, tag="p")
                t_t = t_pool.tile([P, f], mybir.dt.float32, tag="t")
                nc.sync.dma_start(out=p_t, in_=p_src)
                nc.sync.dma_start(out=t_t, in_=t_src)
                t_hi = t_t[:].bitcast(mybir.dt.bfloat16).rearrange(
                    "p (f two) -> p f two", two=2
                )[:, :, 1]
                # Sub-512 tiles contribute one narrow chunk accumulated
                # into the low lanes of the segment bank — the host sums
                # all 512 lanes, so lane balance doesn't matter.
                w = min(MM_N, f)
                for j in range(max(1, f // MM_N)):
                    nc.tensor.matmul(
                        out=psum_seg[s][:, :w],
                        lhsT=ones_t[:, :],
                        rhs=t_hi[:, j * w : (j + 1) * w],
                        start=(mm_done[s] == 0),
                        stop=(mm_done[s] == mm_total[s] - 1),
                    )
                    mm_done[s] += 1
                # d = (p - 1) + t, in place into p_t
                nc.vector.scalar_tensor_tensor(
                    out=p_t,
                    in0=p_t,
                    scalar=1.0,
                    in1=t_t,
                    op0=mybir.AluOpType.subtract,
                    op1=mybir.AluOpType.add,
                )
                if i in SQUARE_TILES:
                    nc.vector.tensor_tensor(
                        out=p_t, in0=p_t, in1=p_t, op=mybir.AluOpType.mult
                    )
                else:
                    nc.scalar.activation(
                        out=p_t, in_=p_t, func=mybir.ActivationFunctionType.Abs
                    )
                nc.scalar.activation(
                    out=p_t,
                    in_=p_t,
                    func=mybir.ActivationFunctionType.Ln,
                    accum_out=vsum_t[:, i : i + 1],
                )
            for s in range(N_SEG):
                if s == N_SEG - 1:
                    # seg2's counts become ready right as DVE runs the
                    # last tiles' STT/TT ladder — a DVE copy wedges into
                    # that in-order chain (+0.7 us measured). ACT has an
                    # idle window there; Copy is in every table set.
                    nc.scalar.activation(
                        out=cnt_sb[:, s * MM_N : (s + 1) * MM_N],
                        in_=psum_seg[s],
                        func=mybir.ActivationFunctionType.Copy,
                    )
                else:
                    nc.vector.tensor_copy(
                        out=cnt_sb[:, s * MM_N : (s + 1) * MM_N],
                        in_=psum_seg[s],
                    )
            # NOTE: DMA cannot read PSUM (dma_start asserts SBUF/DRAM
            # source), so counts bounce through SBUF on DVE. Collapsing
            # vsum's partition dim with an fp32 PE matmul looks
            # attractive (one 60B descriptor instead of 128 tiny ones)
            # but any fp32r matmul in the NEFF statically derates ALL
            # engine clocks by 1.2x (+5 us). Don't.
            # Ship results in readiness order so only a tiny vsum chunk
            # trails the last Ln: bulk vsum columns (ready mid-taper),
            # then counts, then the last taper columns.
            vs_split = NTILES - 3
            nc.sync.dma_start(
                out=vsum_out[:, :vs_split], in_=vsum_t[:, :vs_split]
            )
            nc.sync.dma_start(out=tsum_out[:], in_=cnt_sb)
            nc.sync.dma_start(
                out=vsum_out[:, vs_split:], in_=vsum_t[:, vs_split:]
            )
    nc.compile()
    return nc


def _get_nc():
    global _NC_CACHE
    if _NC_CACHE is None:
        _NC_CACHE = _build_nc()
    return _NC_CACHE


def _run_device(input, target, **spmd_kwargs):
    p_flat = np.ascontiguousarray(input, dtype=np.float32).reshape(-1)
    t_flat = np.ascontiguousarray(target, dtype=np.float32).reshape(-1)
    in_maps = []
    for k in range(N_CORES):
        sl = slice(k * PER_CORE, (k + 1) * PER_CORE)
        in_maps.append({"p_in": p_flat[sl], "t_in": t_flat[sl]})
    return run_bass_kernel_spmd(nc=_get_nc(), in_maps=in_maps,
                                core_ids=list(range(N_CORES)), **spmd_kwargs)


def _epilogue(results):
    sum_v = np.zeros(C, dtype=np.float64)
    sum_t = np.zeros(C, dtype=np.float64)
    for k in range(N_CORES):
        vs = results[k]["vsum"].astype(np.float64)   # [P, NTILES]
        ts = results[k]["tsum"].astype(np.float64)   # [1, N_SEG*MM_N]
        off = 0
        for i, n in enumerate(TILE_ELEMS):
            g = k * PER_CORE + off
            off += n
            ch = (g // SPATIAL) % C
            scale = 0.5 if i in SQUARE_TILES else 1.0
            sum_v[ch] += scale * vs[:, i].sum()
        for s in range(N_SEG):
            ch = ((k * N_SEG + s) // 2) % C
            sum_t[ch] += ts[0, s * MM_N : (s + 1) * MM_N].sum()
    total = float(M_PER_CH)
    ones = sum_t
    weight = np.where(ones > 0, total / np.maximum(ones, 1.0), EMPTY_WEIGHT)
    bce = -sum_v / total
    return np.asarray((weight * bce).mean(), dtype=np.float32)


def kernel(input, target):
    res = _run_device(input, target)
    return _epilogue(res.results)


# revision 47
# speedup vs baseline: 1.1334x; 1.0347x over previous
"""Trainium2 Bass kernel for CustomBCELoss.

Reference semantics (per torch BCELoss with per-channel weighting):
    p, t flattened channel-first to (C=3, M=8388608)
    ones[c]   = count_nonzero(t[c])
    weight[c] = M / max(ones[c], 1)  if ones[c] > 0 else 1000.0
    bce[c]    = -mean(t*max(log p, -100) + (1-t)*max(log1p(-p), -100))
    out       = mean(weight * bce)

Since t ∈ {0,1}, the per-element term is log|p + t - 1|, and with
p ∈ [1e-4, 1-1e-4] (post-sigmoid probabilities) the -100 clamp never
fires: |p + t - 1| >= ~6e-5 so log >= ~-10.

8-way data-parallel over the flat element range. Per-core pipeline over
[128, f] tiles. The stream is DMA-bound: 16 HW DMA engines sustain
~425 GB/s/core when the 8 cores' streams de-overlap (and chip-HBM
fair-share ~360 GB/s when they fully overlap — the run-to-run spread),
so the 25.2 MB p+t stream floors at ~59-70 us. The schedule is built so
nothing else ever gates that stream:
  * tiles open with 1024 (first compute ~2.5 us after the stream
    starts; a string of sub-1MB tiles would starve the engines — each
    Sync DMA_DIRECT2D issue costs ~0.6 us), cruise at 2048, taper
    1024/1024/1024/512/512 so the post-stream drain is short chains on
    small tiles. 256-col taper tiles were tried and are NET-WORSE (the
    two extra issue/semaphore sets cost more than the shorter chain).
  * all tiles are <= 2048 cols -> 1 MB pool slots, bufs=8 per pool
    (16 MB SBUF) gives the in-order Sync issuer ~8 tiles of runway so
    slot recycling never stalls descriptor issue;
  * engine split per tile: Sync issues all DMA; PE does per-segment
    positive counts (t is exactly 0.0f/1.0f, so the strided bf16 view
    of its high 2 bytes is exactly 0.0/1.0 -- single-pass bf16 matmuls
    ones.T @ t into PSUM, exact); DVE computes d = (p - 1) + t (fused
    STT) and u = d*d for SQUARE_TILES (their Ln accumulates 2*log|d|,
    halved on the host); ACT does u = |d| for the remaining tiles and
    Ln(u) in place with fused per-partition accum_out. See the
    SQUARE_TILES comment for the placement rules (engine backlog at
    stream end + the 1.2x clock-derate trap). A dummy Ln in the
    preamble pins the natural_log table set (contains Abs+Square+Ln):
    one load, preamble.
Tiles never cross an (n, c) half-block boundary, so per-tile/per-segment
partials map 1:1 to channels on the host, which applies the tiny
weight/mean epilogue in float64.
"""

import numpy as np

import concourse.bacc as bacc
import concourse.bass as bass
import concourse.tile as tile
from concourse import mybir
from concourse.bass_utils import run_bass_kernel_spmd

N_CORES = 8
C = 3
SPATIAL = 128 * 128 * 128            # elements per (n, c) block
N_BATCH = 4
FULL = N_BATCH * C * SPATIAL         # 25_165_824 total elements
PER_CORE = FULL // N_CORES           # 3_145_728
P = 128
# Per-partition column counts per tile; sum must equal PER_CORE / P = 24576.
# Mild ramp at the start (a string of small tiles would starve the 16 DMA
# engines: each Sync DMA_DIRECT2D issue costs ~0.6 us, so sub-1MB tiles
# can't keep 425 GB/s fed), cruise at 2048, taper at the end (short drain).
TILE_F = [1024, 2048, 2048, 2048, 1024,
          2048, 2048, 2048, 2048,
          2048, 2048, 1024, 1024, 1024, 512, 512]
NTILES = len(TILE_F)
TILE_ELEMS = [P * f for f in TILE_F]
assert sum(TILE_ELEMS) == PER_CORE
# |d| as d*d on DVE for these tiles (rest: Abs on ACT). Placement sets
# each engine's backlog at stream end: tiles 5-8 all abs lets DVE catch
# up mid-stream; tile 9 square lets ACT recover; the taper alternates
# square/abs so BOTH engines share the drain (an all-square taper
# saturates DVE with a serial STT->TT ladder, an all-abs taper does the
# same to ACT with Abs->Ln — each measured ~+4 us).
# CAUTION: strict square/abs alternation across ALL tiles (e.g. evens
# square) makes the NEFF run with every engine clock derated 1.2x
# (+5 us) — some static power model keys on sustained DVE+ACT
# co-activity. Alternation on a handful of tiles with clusters
# elsewhere measures at full clock.
SQUARE_TILES = {0, 2, 4, 9, 11, 13, 15}
HALF_BLOCK_COLS = (SPATIAL // 2) // P          # 8192 cols per half-block
N_SEG = (PER_CORE // P) // HALF_BLOCK_COLS     # 3 segments per core
MM_N = 512                                      # matmul moving free dim
M_PER_CH = FULL // C                 # 8_388_608
EMPTY_WEIGHT = 1000.0

_NC_CACHE = None


def _build_nc():
    nc = bacc.Bacc(
        "TRN2", target_bir_lowering=False, debug=False, num_devices=N_CORES
    )
    p_in = nc.declare_dram_parameter(
        "p_in", [PER_CORE], mybir.dt.float32, isOutput=False
    )
    t_in = nc.declare_dram_parameter(
        "t_in", [PER_CORE], mybir.dt.float32, isOutput=False
    )
    vsum_out = nc.declare_dram_parameter(
        "vsum", [P, NTILES], mybir.dt.float32, isOutput=True
    )
    tsum_out = nc.declare_dram_parameter(
        "tsum", [1, N_SEG * MM_N], mybir.dt.float32, isOutput=True
    )

    seg_of_tile = []
    off = 0
    for f in TILE_F:
        assert off // HALF_BLOCK_COLS == (off + f - 1) // HALF_BLOCK_COLS
        seg_of_tile.append(off // HALF_BLOCK_COLS)
        off += f
    mm_total = {s: 0 for s in range(N_SEG)}
    for i, f in enumerate(TILE_F):
        mm_total[seg_of_tile[i]] += max(1, f // MM_N)

    with tile.TileContext(nc) as tc:
        with (
            tc.tile_pool(name="pp", bufs=8) as p_pool,
            tc.tile_pool(name="tp", bufs=8) as t_pool,
            tc.tile_pool(name="res", bufs=1) as res_pool,
            tc.tile_pool(name="ps", bufs=1, space="PSUM") as ps_pool,
        ):
            ones_t = res_pool.tile([P, 1], mybir.dt.bfloat16)
            nc.vector.memset(ones_t, 1.0)
            vsum_t = res_pool.tile([P, NTILES], mybir.dt.float32)
            cnt_sb = res_pool.tile([1, N_SEG * MM_N], mybir.dt.float32)
            # Dummy Ln pins the natural_log table set (contains Abs too).
            warm_t = res_pool.tile([P, 1], mybir.dt.float32)
            nc.vector.memset(warm_t, 1.0)
            nc.scalar.activation(
                out=warm_t, in_=warm_t, func=mybir.ActivationFunctionType.Ln
            )
            psum_seg = [
                ps_pool.tile(
                    [1, MM_N], mybir.dt.float32, tag=f"seg{s}", name=f"psum_seg{s}"
                )
                for s in range(N_SEG)
            ]
            mm_done = {s: 0 for s in range(N_SEG)}
            off = 0
            for i, f in enumerate(TILE_F):
                n = P * f
                p_src = p_in[off : off + n].rearrange("(p f) -> p f", p=P)
                t_src = t_in[off : off + n].rearrange("(p f) -> p f", p=P)
                off += n
                s = seg_of_tile[i]
                p_t = p_pool.tile([P, f], mybir.dt.float32, tag="p")
                t_t = t_pool.tile([P, f], mybir.dt.float32, tag="t")
                nc.sync.dma_start(out=p_t, in_=p_src)
                nc.sync.dma_start(out=t_t, in_=t_src)
                t_hi = t_t[:].bitcast(mybir.dt.bfloat16).rearrange(
                    "p (f two) -> p f two", two=2
                )[:, :, 1]
                # Sub-512 tiles contribute one narrow chunk accumulated
                # into the low lanes of the segment bank — the host sums
                # all 512 lanes, so lane balance doesn't matter.
                w = min(MM_N, f)
                for j in range(max(1, f // MM_N)):
                    nc.tensor.matmul(
                        out=psum_seg[s][:, :w],
                        lhsT=ones_t[:, :],
                        rhs=t_hi[:, j * w : (j + 1) * w],
                        start=(mm_done[s] == 0),
                        stop=(mm_done[s] == mm_total[s] - 1),
                    )
                    mm_done[s] += 1
                # d = (p - 1) + t, in place into p_t
                nc.vector.scalar_tensor_tensor(
                    out=p_t,
                    in0=p_t,
                    scalar=1.0,
                    in1=t_t,
                    op0=mybir.AluOpType.subtract,
                    op1=mybir.AluOpType.add,
                )
                if i in SQUARE_TILES:
                    nc.vector.tensor_tensor(
                        out=p_t, in0=p_t, in1=p_t, op=mybir.AluOpType.mult
                    )
                else:
                    nc.scalar.activation(
                        out=p_t, in_=p_t, func=mybir.ActivationFunctionType.Abs
                    )
                nc.scalar.activation(
                    out=p_t,
                    in_=p_t,
                    func=mybir.ActivationFunctionType.Ln,
                    accum_out=vsum_t[:, i : i + 1],
                )
            # seg2's counts become ready mid-way through the last tiles'
            # drain ladder; the ~0.65 us PSUM->SBUF copy wedges into one
            # engine's in-order queue no matter where it runs (DVE and
            # ACT variants both measured) — DMA cannot read PSUM, so the
            # bounce is unavoidable.
            for s in range(N_SEG):
                nc.vector.tensor_copy(
                    out=cnt_sb[:, s * MM_N : (s + 1) * MM_N],
                    in_=psum_seg[s],
                )
            # NOTE: DMA cannot read PSUM (dma_start asserts SBUF/DRAM
            # source), so counts bounce through SBUF on DVE. Collapsing
            # vsum's partition dim with an fp32 PE matmul looks
            # attractive (one 60B descriptor instead of 128 tiny ones)
            # but any fp32r matmul in the NEFF statically derates ALL
            # engine clocks by 1.2x (+5 us). Don't.
            # Ship results in readiness order so only a tiny vsum chunk
            # trails the last Ln: bulk vsum columns (ready mid-taper),
            # then counts, then the last taper columns.
            vs_split = NTILES - 3
            nc.sync.dma_start(
                out=vsum_out[:, :vs_split], in_=vsum_t[:, :vs_split]
            )
            nc.sync.dma_start(out=tsum_out[:], in_=cnt_sb)
            nc.sync.dma_start(
                out=vsum_out[:, vs_split:], in_=vsum_t[:, vs_split:]
            )
    nc.compile()
    return nc


def _get_nc():
    global _NC_CACHE
    if _NC_CACHE is None:
        _NC_CACHE = _build_nc()
    return _NC_CACHE


def _run_device(input, target, **spmd_kwargs):
    p_flat = np.ascontiguousarray(input, dtype=np.float32).reshape(-1)
    t_flat = np.ascontiguousarray(target, dtype=np.float32).reshape(-1)
    in_maps = []
    for k in range(N_CORES):
        sl = slice(k * PER_CORE, (k + 1) * PER_CORE)
        in_maps.append({"p_in": p_flat[sl], "t_in": t_flat[sl]})
    return run_bass_kernel_spmd(nc=_get_nc(), in_maps=in_maps,
                                core_ids=list(range(N_CORES)), **spmd_kwargs)


def _epilogue(results):
    sum_v = np.zeros(C, dtype=np.float64)
    sum_t = np.zeros(C, dtype=np.float64)
    for k in range(N_CORES):
        vs = results[k]["vsum"].astype(np.float64)   # [P, NTILES]
        ts = results[k]["tsum"].astype(np.float64)   # [1, N_SEG*MM_N]
        off = 0
        for i, n in enumerate(TILE_ELEMS):
            g = k * PER_CORE + off
            off += n
            ch = (g // SPATIAL) % C
            scale = 0.5 if i in SQUARE_TILES else 1.0
            sum_v[ch] += scale * vs[:, i].sum()
        for s in range(N_SEG):
            ch = ((k * N_SEG + s) // 2) % C
            sum_t[ch] += ts[0, s * MM_N : (s + 1) * MM_N].sum()
    total = float(M_PER_CH)
    ones = sum_t
    weight = np.where(ones > 0, total / np.maximum(ones, 1.0), EMPTY_WEIGHT)
    bce = -sum_v / total
    return np.asarray((weight * bce).mean(), dtype=np.float32)


def kernel(input, target):
    res = _run_device(input, target)
    return _epilogue(res.results)
